# revision 42
# baseline (speedup 1.0000x reference)
"""Grouped SwiGLU experts (MoE, contiguous per-expert token segments) on 8 trn2 cores.

Strategy: expert-parallel over 512-token slots (as the bf16 baseline), but all
matmuls run in fp8-e4m3 with MatmulPerfMode.DoubleRow (K=256 per instruction,
0.5 cycles/row -> 4x bf16 FLOP rate).  Plain fp8 is ~6.5% rel err, far over
the 2e-2 budget, so every tensor is carried as a hi+lo e4m3 pair and each
GEMM computes the significant cross terms
    a@b ~= a_hi@b_hi + a_hi@b_lo + a_lo@b_hi        (lo@lo ~ 1e-3 rel, dropped)
The lo parts keep the SAME scale as their hi parts, so all terms accumulate
into a single PSUM bank with no fixup arithmetic.  A measured slice of the
error budget is traded for cycles: the x-lo correction skips its last
DoubleRow pair (K chunks 14,15) and the h-lo correction skips the hl10
chunk, landing at 1.595e-2 rel err (limit 2e-2) for ~16us.  Net tensor time
~0.72x rows vs 1.0x for bf16.

Scales keep everything in e4m3's happy range: x at 1, w1 at 64, w3 at 16,
w2 at 64; h is produced at scale 16 (= silu(ps1/64) * ps3), output PSUM is
scale 1024 and is written back as bf16 after a 1/1024 scale.

H=1408 is 11 128-chunks (odd), so phase-2 chunk layouts are arranged so all
32 kept product chunks fit in 16 DoubleRow pairs: h is
[hh0..hh10, hh10(dup), hl0..hl10, Z(unused)], w2 is [wh0..wh10, wl10,
wl0..wl9]; the (hh10,hh10')x(wh10,wl10) pair covers both hh10 leftovers.

Schedule notes (cost-model-driven): loads ride the SP queue, a strict
blocking FIFO, in priority order; hg10 is processed mid-phase-1 so its h
writes land before phase 2 reads them; the first three phase-2 chains are
interleaved (their 3 pairs touching the last hid-chunk's writes run last);
~150 tiny scratch matmuls warm the PE pstate through the initial DMA wait.

kernel(**inputs) -> full [16384, 2048] fp32 output.  Self-contained.
"""

import math

import numpy as np
import ml_dtypes

import concourse.bass as bass
import concourse.tile as tile
from concourse import bacc
from concourse import mybir
from concourse.alu_op_type import AluOpType
from concourse.bass_utils import run_bass_kernel_spmd

N_CORES = 8
D = 2048          # dim_in
H = 1408          # dim_hidden
TOK = 512         # tokens per slot
P = 128           # partitions
KC = D // P       # 16 k-chunks over dim_in
HC = H // P       # 11 hid-chunks
TK = TOK // P     # 4 token tiles per slot
NG = 512          # phase-2 out-column group width
NGRP = D // NG    # 4 column groups
NCH = 24          # h chunk axis: [hh*11, hh10dup, hl*11, Z]
NCW = 22          # w2 chunk axis: [wh*11, wl10, wl0..wl9]
WR = 5            # w13 tag rotation depth (SBUF vs prefetch tradeoff)

SW1 = 64.0        # w1 quant scale
SW3 = 16.0        # w3 quant scale -> h comes out at scale 16
SW2 = 64.0        # w2 quant scale
OSC = 1.0 / (16.0 * 64.0)  # final psum -> out scale

F8 = ml_dtypes.float8_e4m3

_compiled_cache = {}
_wq_cache = {}
last_run_info = {}


def _build_program(S):
    """Per-core SPMD program: S slots, each 512 tokens of one expert."""
    nc = bacc.Bacc()
    f8 = mybir.dt.float8e4
    f32 = mybir.dt.float32
    bf16 = mybir.dt.bfloat16
    DRM = mybir.MatmulPerfMode.DoubleRow
    Act = mybir.ActivationFunctionType

    xt = nc.declare_dram_parameter("xt", [S, 2, P, KC, TOK], f8, isOutput=False)
    w13 = nc.declare_dram_parameter("w13", [S, HC, P, 4, KC, P], f8, isOutput=False)
    w2c = nc.declare_dram_parameter("w2c", [S, NGRP, P, NCW, NG], f8, isOutput=False)
    out = nc.declare_dram_parameter("out", [S * TOK, D], bf16, isOutput=True)

    with tile.TileContext(nc) as tc:
        with (
            tc.tile_pool(name="xp", bufs=2) as xp,
            tc.tile_pool(name="wp", bufs=1) as wp,
            tc.tile_pool(name="w2p", bufs=1) as w2p,
            tc.tile_pool(name="hp", bufs=2) as hp,
            tc.tile_pool(name="tp", bufs=2) as tp,
            tc.tile_pool(name="op", bufs=4) as op,
            tc.tile_pool(name="psA", bufs=4, space="PSUM") as psa,
            tc.tile_pool(name="psB", bufs=4, space="PSUM") as psb,
        ):
            # phase-1 processing order: hg10 runs mid-phase so its h chunks
            # (hh10, the dup, hl10) are long done before phase 2 reads them
            # in its final DoubleRow pairs
            PROC = [0, 1, 2, 3, 4, 10, 5, 6, 7, 8, 9]

            # PE pstate warmup: ~150 tiny self-contained matmuls on scratch
            # data keep the PE busy through the initial DMA wait so the
            # first real chains run at full clock
            scr_w = tp.tile([P, 2, P], f8, tag="scrw", bufs=1, name="scr_w")
            scr_x = tp.tile([P, 2, 64], f8, tag="scrx", bufs=1, name="scr_x")
            nc.gpsimd.memset(scr_w[:], 0)
            nc.gpsimd.memset(scr_x[:], 0)
            scr_ps = psa.tile([P, TOK], f32, tag="ps", name="scr_ps")
            for i in range(150):
                nc.tensor.matmul(out=scr_ps[:, 0:64], lhsT=scr_w[:], rhs=scr_x[:],
                                 start=True, stop=True, perf_mode=DRM)

            for s in range(S):
                # ---- loads.  All on the SP queue, which is a strict
                # blocking FIFO (a DMA holds the SEQ during its semaphore
                # waits), so issue order IS priority order.  w13 issues are
                # paced by their tag-rotation frees; by the time the FIFO
                # reaches this slot's w2 issues, the previous slot's phase 2
                # is done, so they never block later loads. ----
                xh = xp.tile([P, KC, TOK], f8, tag="xh", name=f"xh_{s}")
                xl = xp.tile([P, KC, TOK], f8, tag="xl", name=f"xl_{s}")
                wt = {}

                def load_w13(pos, s=s, wt=wt):
                    hg = PROC[pos]
                    t = wp.tile([P, 4, KC, P], f8, tag=f"w_{pos % WR}",
                                name=f"w13_{s}_{hg}")
                    nc.sync.dma_start(out=t[:], in_=w13[s, hg])
                    wt[hg] = t

                KH = KC // 2
                if s == 0:
                    # fine-grained first loads: the first chain can start
                    # after one x quarter + half a w13 kind instead of 2MB
                    t = wp.tile([P, 4, KC, P], f8, tag="w_0", name="w13_0_0")
                    wt[0] = t
                    nc.sync.dma_start(out=t[:, 0, 0:KH, :], in_=w13[0, 0, :, 0, 0:KH])
                    nc.sync.dma_start(out=xh[:, 0:4, :], in_=xt[0, 0, :, 0:4])
                    nc.sync.dma_start(out=t[:, 0, KH:KC, :], in_=w13[0, 0, :, 0, KH:KC])
                    nc.sync.dma_start(out=xh[:, 4:8, :], in_=xt[0, 0, :, 4:8])
                    nc.sync.dma_start(out=t[:, 1], in_=w13[0, 0, :, 1])
                    nc.sync.dma_start(out=xh[:, 8:12, :], in_=xt[0, 0, :, 8:12])
                    nc.sync.dma_start(out=xh[:, 12:16, :], in_=xt[0, 0, :, 12:16])
                    nc.sync.dma_start(out=t[:, 2], in_=w13[0, 0, :, 2])
                    nc.sync.dma_start(out=t[:, 3], in_=w13[0, 0, :, 3])
                    # second tile's w1 kinds land before x-lo so hg1 can
                    # start the moment hg0's x-lo terms finish
                    t1 = wp.tile([P, 4, KC, P], f8, tag="w_1", name="w13_0_1")
                    wt[1] = t1
                    nc.sync.dma_start(out=t1[:, 0:2], in_=w13[0, 1, :, 0:2])
                    # x-lo chunks 14,15 are never used (skipped correction)
                    nc.sync.dma_start(out=xl[:, 0:KH, :], in_=xt[0, 1, :, 0:KH])
                    nc.sync.dma_start(out=xl[:, KH:KC - 2, :],
                                      in_=xt[0, 1, :, KH:KC - 2])
                    nc.sync.dma_start(out=t1[:, 2:4], in_=w13[0, 1, :, 2:4])
                else:
                    nc.sync.dma_start(out=xh[:, 0:KH, :], in_=xt[s, 0, :, 0:KH])
                    load_w13(0)
                    nc.sync.dma_start(out=xh[:, KH:KC, :], in_=xt[s, 0, :, KH:KC])
                    nc.sync.dma_start(out=xl[:, 0:KH, :], in_=xt[s, 1, :, 0:KH])
                    nc.sync.dma_start(out=xl[:, KH:KC - 2, :],
                                      in_=xt[s, 1, :, KH:KC - 2])
                for pos in range(2 if s == 0 else 1, HC):
                    load_w13(pos)
                w2t = []
                for g in range(NGRP):
                    t = w2p.tile([P, NCW, NG], f8, tag=f"w2_{g}",
                                 name=f"w2_{s}_{g}")
                    nc.sync.dma_start(out=t[:], in_=w2c[s, g])
                    w2t.append(t)

                h = hp.tile([P, NCH, TOK], f8, tag="h", name=f"h_{s}")

                # ---- phase 1: h = silu(x@w1) * (x@w3), hi/lo split on chip ----
                for pi, hg in enumerate(PROC):
                    ps1 = psa.tile([P, TOK], f32, tag="ps", name=f"ps1_{s}_{hg}")
                    ps3 = psa.tile([P, TOK], f32, tag="ps", name=f"ps3_{s}_{hg}")
                    w = wt[hg]

                    # x-lo correction covers K chunks 0..13 only (the last
                    # DoubleRow pair is skipped: spends ~1e-2 of the 2e-2
                    # error budget for ~9us)
                    def chain(psx, khi, klo, part):
                        seq = ([(khi, xh, d, n == 0, False)
                                for n, d in enumerate(range(KC // 2))]
                               + [(klo, xh, d, False, False)
                                  for d in range(KC // 2)]
                               + [(khi, xl, d, False, d == KC // 2 - 2)
                                  for d in range(KC // 2 - 1)])
                        lo, hi = (0, 2 * (KC // 2)) if part == 0 else \
                                 (2 * (KC // 2), len(seq)) if part == 1 else \
                                 (0, len(seq))
                        for wk, xx, d, st, sp in seq[lo:hi]:
                            nc.tensor.matmul(
                                out=psx[:], lhsT=w[:, wk, 2 * d:2 * d + 2, :],
                                rhs=xx[:, 2 * d:2 * d + 2, :],
                                start=st, stop=sp, perf_mode=DRM,
                            )

                    if pi < 2:
                        # head of the slot: both hi-term chains first (their
                        # weight tiles arrive before x-lo does on slot 0)
                        chain(ps1, 0, 1, 0)
                        chain(ps3, 2, 3, 0)
                        chain(ps1, 0, 1, 1)
                        chain(ps3, 2, 3, 1)
                    else:
                        chain(ps1, 0, 1, 2)
                        chain(ps3, 2, 3, 2)
                    sil = tp.tile([P, TOK], f32, tag="sil", name=f"sil_{s}_{hg}")
                    nc.scalar.activation(sil[:], ps1[:], Act.Silu, scale=1.0 / SW1)
                    h16 = tp.tile([P, TOK], f32, tag="h16", name=f"h16_{s}_{hg}")
                    nc.vector.tensor_tensor(out=h16[:], in0=sil[:], in1=ps3[:],
                                            op=AluOpType.mult)
                    nc.gpsimd.tensor_copy(out=h[:, hg, :], in_=h16[:])
                    if hg == HC - 1:  # duplicate hh10 for the leftover pair
                        nc.gpsimd.tensor_copy(out=h[:, HC, :], in_=h16[:])
                    else:  # hl10 is unused (its correction pair is dropped)
                        nc.vector.tensor_tensor(out=h[:, HC + 1 + hg, :],
                                                in0=h16[:], in1=h[:, hg, :],
                                                op=AluOpType.subtract)

                # ---- phase 2: out = (h_hi+h_lo) @ (w2_hi+w2_lo), 3 terms
                # covered by 17 DoubleRow pairs (see module docstring) ----
                HL = HC + 1  # h-lo chunk base (12)
                WL = HC + 1  # w2-lo chunk base (12); wl10 sits at 11
                # pair order: the 14 pairs whose h chunks are written by
                # mid-phase-1 come first; the 3 pairs touching the last
                # processed hid-chunk's writes (hh8/hh9 cast, hl8/hl9 sub)
                # come last, so phase 2 can start before phase 1's tail
                # elementwise ops land
                # the (hl10,Z) pair is dropped: hl10's correction is worth
                # ~4e-3 of error budget and a full DR per chain
                p2_pairs = (
                    [(2 * c, 2 * c) for c in range(4)]            # main c0-3
                    + [(2 * c, WL + 2 * c) for c in range(4)]     # w2-lo c0-3
                    + [(HL + 2 * c, 2 * c) for c in range(4)]     # h-lo c0-3
                    + [(HC - 1, HC - 1)]                          # hh10 x (wh10,wl10)
                    + [(8, 8), (8, WL + 8)]                       # main/w2-lo c4 (hh9)
                    + [(HL + 8, 8)]                               # h-lo c4 (hl9)
                )
                NEARLY = 13  # pairs with no dependency on the last hg's writes

                def p2_chain(ci, lo, hi, pso):
                    g, tk = ci // TK, ci % TK
                    for n in range(lo, hi):
                        hc, wc = p2_pairs[n]
                        nc.tensor.matmul(
                            out=pso[:],
                            lhsT=h[:, hc:hc + 2, tk * P:(tk + 1) * P],
                            rhs=w2t[g][:, wc:wc + 2, :],
                            start=(n == 0),
                            stop=(n == len(p2_pairs) - 1),
                            perf_mode=DRM,
                        )

                def p2_finish(ci, pso, o_sb):
                    g, tk = ci // TK, ci % TK
                    last = (ci == NGRP * TK - 1)
                    if g % 2 == 0:
                        o_sb[tk] = op.tile([P, 2 * NG], bf16, tag=f"o_{tk}",
                                           bufs=2, name=f"o_{s}_{g // 2}_{tk}")
                    o = o_sb[tk]
                    half = o[:, (g % 2) * NG:(g % 2 + 1) * NG]
                    rows = slice(s * TOK + tk * P, s * TOK + (tk + 1) * P)
                    if last:
                        # final chain of the slot: copy AND store both via
                        # the Act queue — same-engine ordering needs no
                        # cross-engine semaphore hops, shortening the tail
                        nc.scalar.activation(o[:, NG:2 * NG], pso[:],
                                             Act.Copy, scale=OSC)
                        nc.scalar.dma_start(
                            out=out[rows, (2 * (g // 2) + 1) * NG:
                                    (2 * (g // 2) + 2) * NG],
                            in_=o[:, NG:2 * NG],
                        )
                        return
                    if ci % 2 == 0:
                        nc.vector.tensor_scalar_mul(out=half, in0=pso[:],
                                                    scalar1=OSC)
                    else:
                        nc.scalar.activation(half, pso[:], Act.Copy, scale=OSC)
                    if g % 2 == 1:
                        nc.gpsimd.dma_start(
                            out=out[rows,
                                    (g // 2) * 2 * NG:(g // 2 + 1) * 2 * NG],
                            in_=o[:],
                        )
                    elif g == NGRP - 2 and tk == TK - 1:
                        # the last chain (g3,tk3) stores its own half; ship
                        # this g2 half now instead of pairing with it
                        nc.gpsimd.dma_start(
                            out=out[rows, (g // 2) * 2 * NG:
                                    (g // 2) * 2 * NG + NG],
                            in_=o[:, 0:NG],
                        )

                o_sb = {}
                NP2 = len(p2_pairs)
                # first three chains interleaved: their early pairs run while
                # the last hg's h writes land, then their tails complete
                first = [psb.tile([P, NG], f32, tag="ps", name=f"pso_{s}_{ci}")
                         for ci in range(3)]
                for ci in range(3):
                    p2_chain(ci, 0, NEARLY, first[ci])
                for ci in range(3):
                    p2_chain(ci, NEARLY, NP2, first[ci])
                    p2_finish(ci, first[ci], o_sb)
                for ci in range(3, NGRP * TK):
                    pso = psb.tile([P, NG], f32, tag="ps", name=f"pso_{s}_{ci}")
                    p2_chain(ci, 0, NP2, pso)
                    p2_finish(ci, pso, o_sb)
    nc.compile()
    return nc


def _plan(m_sizes, T):
    """Mirror the reference routing: contiguous segments by expert, then chop
    into TOK-sized chunks and deal them contiguously across cores."""
    bounds = np.cumsum(np.asarray(m_sizes, dtype=np.int64))
    E = len(bounds)
    chunks = []  # (expert, row_start, nrows)
    prev = 0
    for e in range(E):
        lo, hi = prev, min(int(bounds[e]), T)
        prev = max(lo, hi)
        seg = hi - lo
        off = lo
        while seg > 0:
            take = min(TOK, seg)
            chunks.append((e, off, take))
            off += take
            seg -= take
    S = max(1, math.ceil(len(chunks) / N_CORES))
    while len(chunks) < N_CORES * S:
        chunks.append((0, 0, 0))  # dummy slot
    per_core = [chunks[c * S:(c + 1) * S] for c in range(N_CORES)]
    return per_core, S


def _hilo(a):
    hi = a.astype(F8)
    lo = (a - hi.astype(np.float32)).astype(F8)
    return hi, lo


def _quant_weights(w1, w2, w3):
    """Per-expert hi/lo fp8 weights in the on-device layouts."""
    E = w1.shape[0]
    w13_e = np.empty((E, HC, P, 4, KC, P), dtype=F8)
    w2_e = np.empty((E, NGRP, P, NCW, NG), dtype=F8)

    def t13(a):  # [D, H] -> [HC, P(k), KC, P(h)]
        return a.reshape(KC, P, HC, P).transpose(2, 1, 0, 3)

    def t2(a):  # [H, D] -> [NGRP, P(h), HC, NG]
        return a.reshape(HC, P, NGRP, NG).transpose(2, 1, 0, 3)

    for e in range(E):
        h1, l1 = _hilo(w1[e] * SW1)
        h3, l3 = _hilo(w3[e] * SW3)
        w13_e[e, :, :, 0] = t13(h1)
        w13_e[e, :, :, 1] = t13(l1)
        w13_e[e, :, :, 2] = t13(h3)
        w13_e[e, :, :, 3] = t13(l3)
        h2, l2 = _hilo(w2[e] * SW2)
        th, tl = t2(h2), t2(l2)
        w2_e[e, :, :, 0:HC] = th          # wh0..wh10
        w2_e[e, :, :, HC] = tl[:, :, HC - 1]   # wl10
        w2_e[e, :, :, HC + 1:NCW] = tl[:, :, 0:HC - 1]  # wl0..wl9
    return w13_e, w2_e


def kernel(x, w1, w2, w3, m_sizes, _trace=False):
    x = np.asarray(x, dtype=np.float32)
    w1 = np.asarray(w1, dtype=np.float32)
    w2 = np.asarray(w2, dtype=np.float32)
    w3 = np.asarray(w3, dtype=np.float32)
    T = x.shape[0]
    assert x.shape[1] == D and w1.shape[1:] == (D, H), (x.shape, w1.shape)
    assert w2.shape[1:] == (H, D) and w3.shape[1:] == (D, H), (w2.shape, w3.shape)

    per_core, S = _plan(m_sizes, T)

    if S not in _compiled_cache:
        _compiled_cache[S] = _build_program(S)
    nc = _compiled_cache[S]

    wkey = (id(w1), id(w2), id(w3))
    if wkey not in _wq_cache:
        _wq_cache.clear()
        _wq_cache[wkey] = _quant_weights(w1, w2, w3)
    w13_e, w2_e = _wq_cache[wkey]

    in_maps = []
    for c in range(N_CORES):
        slots = per_core[c]
        xt_c = np.zeros((S, 2, P, KC, TOK), dtype=F8)
        for s, (e, off, ln) in enumerate(slots):
            if ln:
                seg = np.zeros((TOK, D), dtype=np.float32)
                seg[:ln] = x[off:off + ln]
                sh, sl = _hilo(seg)
                # [TOK, D] -> [P(k), KC, TOK]
                xt_c[s, 0] = sh.reshape(TOK, KC, P).transpose(2, 1, 0)
                xt_c[s, 1] = sl.reshape(TOK, KC, P).transpose(2, 1, 0)
        eids = [e for (e, _, _) in slots]
        in_maps.append({
            "xt": xt_c,
            "w13": np.ascontiguousarray(w13_e[eids]),
            "w2c": np.ascontiguousarray(w2_e[eids]),
        })

    try:
        res = run_bass_kernel_spmd(
            nc, in_maps, list(range(N_CORES)), trace=_trace,
        )
    except Exception:
        # transient NRT device errors have been observed once after a fresh
        # compile; a single retry is free if the device truly died
        res = run_bass_kernel_spmd(
            nc, in_maps, list(range(N_CORES)), trace=_trace,
        )

    full = np.zeros((T, D), dtype=np.float32)
    for c in range(N_CORES):
        oc = res.results[c]["out"].astype(np.float32)
        for s, (e, off, ln) in enumerate(per_core[c]):
            if ln:
                full[off:off + ln] = oc[s * TOK:s * TOK + ln]

    last_run_info.clear()
    last_run_info.update({
        "exec_time_ns": res.exec_time_ns,
        "profile_json": getattr(res, "profile_json", None),
        "S": S,
    })
    return full
